# revision 3
# baseline (speedup 1.0000x reference)
"""Multi-head self-attention (B=4, S=1024, D=1024, H=16, RoPE, causal) on 8
Trainium2 NeuronCores.

Sharding: 8 cores = 4 batches x 2 head-groups (8 heads each). Each core
computes QKV projections for its batch/head-group, RoPE, causal attention,
and a partial output projection (contraction over its 512 attention dims).
The host sums the two partial outputs per batch (the "all-reduce") and
concatenates batches.

Device layout notes:
- Weights are passed transposed; Q/K projection output dims are permuted to
  rotate-half order (evens then odds within each head) so RoPE works on
  contiguous 32-column halves. Permuting Q and K identically leaves Q.K^T
  unchanged.
- Logits are computed transposed (L^T[k, q]) so softmax sums reduce over the
  PSUM partition axis via a ones-column appended to V, and the attention
  output arrives as attn^T[c, q] which feeds the output projection directly.
- The final output is produced transposed (y^T[o, q]); the host transposes.
"""

import numpy as np

import concourse.bass as bass
import concourse.mybir as mybir
import concourse.tile as tile
from concourse.bass import ts
from concourse.bass_utils import run_bass_kernel_spmd
from concourse.masks import make_identity, make_upper_triangular

B, S, D = 4, 1024, 1024
H = 16  # total heads
HG = 8  # heads per core (head-group)
DK = 64  # head dim
DG = HG * DK  # 512, per-core projection width
ROPE_THETA = 10000.0
P = 128  # partitions
NS = S // P  # 8 s-tiles
ND = D // P  # 8 d-chunks
F32 = mybir.dt.float32

_uid = [0]


def _split_excess_waits(nc, limit=1):
    """This container's walrus rejects >1 sync waits on the kernel-tail
    Drain; move excess waits onto same-engine NoOps inserted before it."""
    for f in nc.m.functions:
        for blk in f.blocks:
            insts = list(blk.instructions)
            out = []
            changed = False
            for inst in insts:
                si = inst.sync_info
                if si is not None and si.on_wait and len(si.on_wait) > limit:
                    waits = list(si.on_wait)
                    head, tail = waits[:-limit], waits[-limit:]
                    for i in range(0, len(head), limit):
                        _uid[0] += 1
                        nop = mybir.InstNoOp(
                            name=f"waitsplit-{_uid[0]}", ins=[], outs=[]
                        )
                        nop.engine = inst.engine
                        nop.sync_info = mybir.SyncInfo(
                            on_wait=head[i : i + limit], on_update=[]
                        )
                        out.append(nop)
                    si.on_wait = tail
                    changed = True
                out.append(inst)
            if changed:
                blk.instructions = out
    return nc


def build_nc():
    nc = bass.Bass("TRN2")
    xT = nc.dram_tensor("xT", [D, S], F32, kind="ExternalInput")
    wqT = nc.dram_tensor("wqT", [D, DG], F32, kind="ExternalInput")
    wkT = nc.dram_tensor("wkT", [D, DG], F32, kind="ExternalInput")
    wvT = nc.dram_tensor("wvT", [D, DG], F32, kind="ExternalInput")
    woT = nc.dram_tensor("woT", [DG, D], F32, kind="ExternalInput")
    cos8 = nc.dram_tensor("cos8", [S, HG * 32], F32, kind="ExternalInput")
    sin8 = nc.dram_tensor("sin8", [S, HG * 32], F32, kind="ExternalInput")
    yT = nc.dram_tensor("yT", [D, S], F32, kind="ExternalOutput")

    with tile.TileContext(nc) as tc:
        with (
            tc.tile_pool(name="const", bufs=1) as constp,
            tc.tile_pool(name="wq", bufs=1) as wqp,
            tc.tile_pool(name="big", bufs=1) as bigp,
        ):
            # constants
            ident = constp.tile([P, P], F32, tag="ident")
            make_identity(nc, ident[:, :])
            tril = constp.tile([P, P], F32, tag="tril")
            make_upper_triangular(nc, tril[:, :], val=1.0, diag=True)
            ones = constp.tile([P, DK], F32, tag="ones")
            nc.vector.memset(ones[:, :], 1.0)

            # resident weights
            wq_sb = [wqp.tile([P, DG], F32, tag=f"wq{c}", name=f"wq{c}") for c in range(ND)]
            wk_sb = [wqp.tile([P, DG], F32, tag=f"wk{c}", name=f"wk{c}") for c in range(ND)]
            wv_sb = [wqp.tile([P, DG], F32, tag=f"wv{c}", name=f"wv{c}") for c in range(ND)]
            wo_sb = [wqp.tile([P, D], F32, tag=f"wo{c}", name=f"wo{c}") for c in range(DG // P)]
            for c in range(ND):
                nc.sync.dma_start(out=wq_sb[c][:, :], in_=wqT[ts(c, P), :])
                nc.sync.dma_start(out=wk_sb[c][:, :], in_=wkT[ts(c, P), :])
                nc.sync.dma_start(out=wv_sb[c][:, :], in_=wvT[ts(c, P), :])
            for c in range(DG // P):
                nc.sync.dma_start(out=wo_sb[c][:, :], in_=woT[ts(c, P), :])

            # persistent activations
            qt_sb = [bigp.tile([P, S], F32, tag=f"qt{p}", name=f"qt{p}") for p in range(4)]
            kt_sb = [bigp.tile([P, S], F32, tag=f"kt{p}", name=f"kt{p}") for p in range(4)]
            v_sb = [bigp.tile([P, HG, DK + 1], F32, tag=f"v{j}", name=f"v{j}") for j in range(NS)]
            at_sb = [bigp.tile([P, S], F32, tag=f"at{p}", name=f"at{p}") for p in range(4)]

            # ---------------- Phase A: projections + RoPE + transposes ----
            with (
                tc.tile_pool(name="pa_psum", bufs=2, space="PSUM") as pap,
                tc.tile_pool(name="tp_psum", bufs=2, space="PSUM") as tpp,
                tc.tile_pool(name="pa_sbuf", bufs=3) as pas,
                tc.tile_pool(name="rope", bufs=4) as ropep,
            ):
                for i in range(NS):
                    xt = [pas.tile([P, P], F32, tag="xt", name="xt") for _ in range(ND)]
                    for c in range(ND):
                        nc.sync.dma_start(
                            out=xt[c][:, :], in_=xT[ts(c, P), ts(i, P)]
                        )
                    cs = pas.tile([P, HG * 32], F32, tag="cos")
                    sn = pas.tile([P, HG * 32], F32, tag="sin")
                    nc.sync.dma_start(out=cs[:, :], in_=cos8[ts(i, P), :])
                    nc.sync.dma_start(out=sn[:, :], in_=sin8[ts(i, P), :])

                    qp = pap.tile([P, DG], F32, tag="q")
                    kp = pap.tile([P, DG], F32, tag="k")
                    vp = pap.tile([P, DG], F32, tag="v")
                    for c in range(ND):
                        st = c == 0
                        sp = c == ND - 1
                        nc.tensor.matmul(
                            qp[:, :], lhsT=xt[c][:, :], rhs=wq_sb[c][:, :],
                            start=st, stop=sp,
                        )
                        nc.tensor.matmul(
                            kp[:, :], lhsT=xt[c][:, :], rhs=wk_sb[c][:, :],
                            start=st, stop=sp,
                        )
                        nc.tensor.matmul(
                            vp[:, :], lhsT=xt[c][:, :], rhs=wv_sb[c][:, :],
                            start=st, stop=sp,
                        )

                    # V -> SBUF with a ones column per head (softmax sums)
                    nc.scalar.copy(
                        out=v_sb[i][:, :, 0:DK],
                        in_=vp[:, :].rearrange("p (h c) -> p h c", h=HG),
                    )
                    nc.vector.memset(v_sb[i][:, :, DK : DK + 1], 1.0)

                    # RoPE on q/k (rotate-half layout: per head [32 even|32 odd])
                    cs3 = cs[:, :].rearrange("p (h c) -> p h c", h=HG)
                    sn3 = sn[:, :].rearrange("p (h c) -> p h c", h=HG)
                    for src, dst_tag in ((qp, "qr"), (kp, "kr")):
                        sv = src[:, :].rearrange(
                            "p (h t c) -> p h t c", h=HG, t=2
                        )
                        ev, od = sv[:, :, 0, :], sv[:, :, 1, :]
                        r = pas.tile([P, DG], F32, tag=dst_tag, name=dst_tag)
                        rv = r[:, :].rearrange("p (h t c) -> p h t c", h=HG, t=2)
                        t1 = ropep.tile([P, HG * 32], F32, tag="t1")
                        t2 = ropep.tile([P, HG * 32], F32, tag="t2")
                        t13 = t1[:, :].rearrange("p (h c) -> p h c", h=HG)
                        t23 = t2[:, :].rearrange("p (h c) -> p h c", h=HG)
                        nc.vector.tensor_mul(t13, ev, cs3)
                        nc.vector.tensor_mul(t23, od, sn3)
                        nc.vector.tensor_sub(rv[:, :, 0, :], t13, t23)
                        t3 = ropep.tile([P, HG * 32], F32, tag="t3")
                        t4 = ropep.tile([P, HG * 32], F32, tag="t4")
                        t33 = t3[:, :].rearrange("p (h c) -> p h c", h=HG)
                        t43 = t4[:, :].rearrange("p (h c) -> p h c", h=HG)
                        nc.vector.tensor_mul(t33, ev, sn3)
                        nc.vector.tensor_mul(t43, od, cs3)
                        nc.vector.tensor_add(rv[:, :, 1, :], t33, t43)

                        # transpose head-pairs into [d, s] tiles
                        dst_tiles = qt_sb if dst_tag == "qr" else kt_sb
                        for p in range(4):
                            tp = tpp.tile([P, P], F32, tag="tp")
                            nc.tensor.transpose(
                                tp[:, :], r[:, ts(p, P)], ident[:, :]
                            )
                            nc.scalar.copy(
                                out=dst_tiles[p][:, ts(i, P)], in_=tp[:, :]
                            )

            # ---------------- Phase B: attention per head ------------------
            with (
                tc.tile_pool(name="attn_psum", bufs=2, space="PSUM") as atp,
                tc.tile_pool(name="lg_psum", bufs=2, space="PSUM") as lgp,
                tc.tile_pool(name="pt_pool", bufs=3) as ptp,
                tc.tile_pool(name="sm_pool", bufs=2) as smp,
            ):
                for h in range(HG):
                    pair, poff = h // 2, 64 * (h % 2)
                    ap = atp.tile([DK + 1, S], F32, tag="attn")
                    for j in range(NS):
                        q0 = P * j
                        lq = S - q0
                        lg = lgp.tile([P, S], F32, tag="lg")
                        for qc in range(q0, S, 512):
                            n = min(512, S - qc)
                            nc.tensor.matmul(
                                lg[:, qc - q0 : qc - q0 + n],
                                lhsT=kt_sb[pair][poff : poff + DK, ts(j, P)],
                                rhs=qt_sb[pair][poff : poff + DK, qc : qc + n],
                                start=True, stop=True,
                            )
                        pt = ptp.tile([P, S], F32, tag="pt")
                        nc.scalar.activation(
                            out=pt[:, 0:lq], in_=lg[:, 0:lq],
                            func=mybir.ActivationFunctionType.Exp,
                            scale=0.125,
                        )
                        nc.vector.tensor_mul(pt[:, 0:P], pt[:, 0:P], tril[:, :])
                        for c0 in range(0, S, 512):
                            lo = max(q0, c0)
                            hi = c0 + 512
                            if lo >= hi:
                                continue
                            nc.tensor.matmul(
                                ap[:, lo:hi],
                                lhsT=v_sb[j][:, h, :],
                                rhs=pt[:, lo - q0 : hi - q0],
                                start=(j == 0), stop=(j == NS - 1),
                                skip_group_check=True,
                            )
                    # softmax normalization: recip of sums, broadcast, scale
                    rc = smp.tile([P, S], F32, tag="recip")
                    nc.vector.reciprocal(out=rc[64:65, :], in_=ap[DK : DK + 1, :])
                    bc = lgp.tile([DK, S], F32, tag="lg")
                    for qc in range(0, S, 512):
                        nc.tensor.matmul(
                            bc[:, qc : qc + 512],
                            lhsT=ones[64:65, 0:DK],
                            rhs=rc[64:65, qc : qc + 512],
                            start=True, stop=True,
                        )
                    bcs = smp.tile([DK, S], F32, tag="bcs")
                    nc.scalar.copy(out=bcs[:, :], in_=bc[:, :])
                    if poff == 0:
                        nc.vector.tensor_mul(
                            at_sb[pair][0:DK, :], ap[0:DK, :], bcs[:, :]
                        )
                    else:
                        tmp = smp.tile([DK, S], F32, tag="odd")
                        nc.vector.tensor_mul(tmp[:, :], ap[0:DK, :], bcs[:, :])
                        nc.sync.dma_start(
                            out=at_sb[pair][DK:P, :], in_=tmp[:, :]
                        )

            # ---------------- Phase D: output projection -------------------
            with (
                tc.tile_pool(name="y_psum", bufs=2, space="PSUM") as yp,
                tc.tile_pool(name="y_sbuf", bufs=3) as ys,
            ):
                for o in range(ND):
                    ypt = yp.tile([P, S], F32, tag="y")
                    for qc in range(0, S, 512):
                        for c in range(DG // P):
                            nc.tensor.matmul(
                                ypt[:, qc : qc + 512],
                                lhsT=wo_sb[c][:, ts(o, P)],
                                rhs=at_sb[c][:, qc : qc + 512],
                                start=(c == 0), stop=(c == DG // P - 1),
                            )
                    ysb = ys.tile([P, S], F32, tag="ysb")
                    nc.scalar.copy(out=ysb[:, :], in_=ypt[:, :])
                    nc.sync.dma_start(out=yT[ts(o, P), :], in_=ysb[:, :])

    _split_excess_waits(nc)
    return nc


_NC_CACHE = {}


def _get_nc():
    if "nc" not in _NC_CACHE:
        _NC_CACHE["nc"] = build_nc()
    return _NC_CACHE["nc"]


# rotate-half permutation within each head: evens then odds
_PERM = np.concatenate([np.arange(0, DK, 2), np.arange(1, DK, 2)])


def _host_prep(x, Wq, Wk, Wv, Wo, token_positions):
    """Build the 8 per-core input dicts."""
    inv_freq = 1.0 / (ROPE_THETA ** (np.arange(0, DK, 2, dtype=np.float32) / DK))
    in_maps = []
    for core in range(8):
        b, g = core // 2, core % 2
        heads = np.arange(HG * g, HG * (g + 1))
        rows_qk = (heads[:, None] * DK + _PERM[None, :]).reshape(-1)
        rows_v = (heads[:, None] * DK + np.arange(DK)[None, :]).reshape(-1)
        pos = token_positions[b].astype(np.float32)  # [S]
        ang = pos[:, None] * inv_freq[None, :]  # [S, 32]
        cos8 = np.tile(np.cos(ang), (1, HG)).astype(np.float32)
        sin8 = np.tile(np.sin(ang), (1, HG)).astype(np.float32)
        in_maps.append(
            {
                "xT": np.ascontiguousarray(x[b].T),
                "wqT": np.ascontiguousarray(Wq[rows_qk, :].T),
                "wkT": np.ascontiguousarray(Wk[rows_qk, :].T),
                "wvT": np.ascontiguousarray(Wv[rows_v, :].T),
                "woT": np.ascontiguousarray(Wo[:, rows_v].T),
                "cos8": cos8,
                "sin8": sin8,
            }
        )
    return in_maps


def kernel(x, Wq, Wk, Wv, Wo, token_positions, _trace=False):
    x = np.asarray(x, dtype=np.float32)
    Wq = np.asarray(Wq, dtype=np.float32)
    Wk = np.asarray(Wk, dtype=np.float32)
    Wv = np.asarray(Wv, dtype=np.float32)
    Wo = np.asarray(Wo, dtype=np.float32)
    token_positions = np.asarray(token_positions)

    nc = _get_nc()
    in_maps = _host_prep(x, Wq, Wk, Wv, Wo, token_positions)
    res = run_bass_kernel_spmd(nc, in_maps, core_ids=list(range(8)), trace=_trace)
    if _trace:
        kernel.last_exec_time_ns = res.exec_time_ns
        kernel.last_results = res

    y = np.empty((B, S, D), dtype=np.float32)
    for b in range(B):
        yT0 = res.results[2 * b]["yT"]
        yT1 = res.results[2 * b + 1]["yT"]
        y[b] = (yT0 + yT1).T
    return y


# revision 8
# speedup vs baseline: 1.9642x; 1.9642x over previous
"""Multi-head self-attention (B=4, S=1024, D=1024, H=16, RoPE, causal) on 8
Trainium2 NeuronCores.

Sharding: 8 cores = 4 batches x 2 head-groups (8 heads each). Each core
computes QKV projections for its batch/head-group, RoPE, causal attention,
and a partial output projection (contraction over its 512 attention dims).
The host sums the two partial outputs per batch (the "all-reduce") and
concatenates batches.

Device layout notes:
- Weights are passed transposed; Q/K projection output dims are permuted to
  rotate-half order (evens then odds within each head) so RoPE works on
  contiguous 32-column halves. Permuting Q and K identically leaves Q.K^T
  unchanged.
- Logits are computed transposed (L^T[k, q]) so softmax sums reduce over the
  PSUM partition axis via a ones-column appended to V, and the attention
  output arrives as attn^T[c, q] which feeds the output projection directly.
- Matmul operands are float32r (1.5 cyc/row vs 4 for fp32, ~1e-4 rel err).
- Softmax 1/sum is computed on a [8,128] reshape (DMA) and broadcast across
  partitions via a DRAM round-trip, keeping the PE out of it.
- The final output is produced transposed (y^T[o, q]); the host transposes.
"""

import numpy as np

import concourse.bass as bass
import concourse.mybir as mybir
import concourse.tile as tile
from concourse.bass import ts
from concourse.bass_utils import run_bass_kernel_spmd
from concourse.masks import make_identity, make_upper_triangular

B, S, D = 4, 1024, 1024
H = 16  # total heads
HG = 8  # heads per core (head-group)
DK = 64  # head dim
DG = HG * DK  # 512, per-core projection width
ROPE_THETA = 10000.0
P = 128  # partitions
NS = S // P  # 8 s-tiles
ND = D // P  # 8 d-chunks
F32 = mybir.dt.float32
F32R = mybir.dt.float32r

_uid = [0]


def _split_excess_waits(nc, limit=1):
    """This container's walrus rejects >1 sync waits on the kernel-tail
    Drain; move excess waits onto same-engine NoOps inserted before it."""
    for f in nc.m.functions:
        for blk in f.blocks:
            insts = list(blk.instructions)
            out = []
            changed = False
            for inst in insts:
                si = inst.sync_info
                if si is not None and si.on_wait and len(si.on_wait) > limit:
                    waits = list(si.on_wait)
                    head, tail = waits[:-limit], waits[-limit:]
                    for i in range(0, len(head), limit):
                        _uid[0] += 1
                        nop = mybir.InstNoOp(
                            name=f"waitsplit-{_uid[0]}", ins=[], outs=[]
                        )
                        nop.engine = inst.engine
                        nop.sync_info = mybir.SyncInfo(
                            on_wait=head[i : i + limit], on_update=[]
                        )
                        out.append(nop)
                    si.on_wait = tail
                    changed = True
                out.append(inst)
            if changed:
                blk.instructions = out
    return nc


def build_nc():
    nc = bass.Bass("TRN2")
    MMD = F32R  # matmul operand dtype
    xT = nc.dram_tensor("xT", [D, S], MMD, kind="ExternalInput")
    wqT = nc.dram_tensor("wqT", [D, DG], MMD, kind="ExternalInput")
    wkT = nc.dram_tensor("wkT", [D, DG], MMD, kind="ExternalInput")
    wvT = nc.dram_tensor("wvT", [D, DG], MMD, kind="ExternalInput")
    woT = nc.dram_tensor("woT", [DG, D], MMD, kind="ExternalInput")
    cos8 = nc.dram_tensor("cos8", [S, HG * 32], F32, kind="ExternalInput")
    sin8 = nc.dram_tensor("sin8", [S, HG * 32], F32, kind="ExternalInput")
    yT = nc.dram_tensor("yT", [D, S], F32, kind="ExternalOutput")
    # DRAM scratch for the softmax 1/sum partition-broadcast round-trip
    rsum = nc.dram_tensor("rsum", [HG, S], F32)
    rrec = nc.dram_tensor("rrec", [HG, S], F32)

    with tile.TileContext(nc) as tc:
        with (
            tc.tile_pool(name="const", bufs=1) as constp,
            tc.tile_pool(name="wq", bufs=1) as wqp,
            tc.tile_pool(name="big", bufs=1) as bigp,
        ):
            # constants (f32r tiles can't be memset directly; build in f32
            # and convert via ACT copy, which rounds)
            identf = constp.tile([P, P], F32, tag="identf")
            make_identity(nc, identf[:, :])
            ident = constp.tile([P, P], MMD, tag="ident")
            nc.scalar.copy(out=ident[:, :], in_=identf[:, :])
            tril = constp.tile([P, P], F32, tag="tril")
            make_upper_triangular(nc, tril[:, :], val=1.0, diag=True)
            onesf = constp.tile([P, HG], F32, tag="onesf")
            nc.vector.memset(onesf[:, :], 1.0)

            # resident weights
            wq_sb = [wqp.tile([P, DG], MMD, tag=f"wq{c}", name=f"wq{c}") for c in range(ND)]
            wk_sb = [wqp.tile([P, DG], MMD, tag=f"wk{c}", name=f"wk{c}") for c in range(ND)]
            wv_sb = [wqp.tile([P, DG], MMD, tag=f"wv{c}", name=f"wv{c}") for c in range(ND)]
            wo_sb = [wqp.tile([P, D], MMD, tag=f"wo{c}", name=f"wo{c}") for c in range(DG // P)]
            for c in range(ND):
                nc.sync.dma_start(out=wq_sb[c][:, :], in_=wqT[ts(c, P), :])
                nc.sync.dma_start(out=wk_sb[c][:, :], in_=wkT[ts(c, P), :])
                nc.sync.dma_start(out=wv_sb[c][:, :], in_=wvT[ts(c, P), :])
            for c in range(DG // P):
                nc.sync.dma_start(out=wo_sb[c][:, :], in_=woT[ts(c, P), :])

            # persistent activations
            qt_sb = [bigp.tile([P, S], MMD, tag=f"qt{p}", name=f"qt{p}") for p in range(4)]
            kt_sb = [bigp.tile([P, S], MMD, tag=f"kt{p}", name=f"kt{p}") for p in range(4)]
            v_sb = [bigp.tile([P, HG, DK + 1], MMD, tag=f"v{j}", name=f"v{j}") for j in range(NS)]
            at_sb = [bigp.tile([P, S], MMD, tag=f"at{p}", name=f"at{p}") for p in range(4)]

            # ---------------- Phase A: projections + RoPE + transposes ----
            with (
                tc.tile_pool(name="pa_psum", bufs=2, space="PSUM") as pap,
                tc.tile_pool(name="tp_psum", bufs=2, space="PSUM") as tpp,
                tc.tile_pool(name="pa_sbuf", bufs=3) as pas,
                tc.tile_pool(name="rope", bufs=4) as ropep,
            ):
                for i in range(NS):
                    xt = [pas.tile([P, P], MMD, tag="xt", name="xt") for _ in range(ND)]
                    for c in range(ND):
                        nc.sync.dma_start(
                            out=xt[c][:, :], in_=xT[ts(c, P), ts(i, P)]
                        )
                    cs = pas.tile([P, HG * 32], F32, tag="cos")
                    sn = pas.tile([P, HG * 32], F32, tag="sin")
                    nc.sync.dma_start(out=cs[:, :], in_=cos8[ts(i, P), :])
                    nc.sync.dma_start(out=sn[:, :], in_=sin8[ts(i, P), :])

                    qp = pap.tile([P, DG], F32, tag="q")
                    kp = pap.tile([P, DG], F32, tag="k")
                    vp = pap.tile([P, DG], F32, tag="v")
                    for c in range(ND):
                        st = c == 0
                        sp = c == ND - 1
                        nc.tensor.matmul(
                            qp[:, :], lhsT=xt[c][:, :], rhs=wq_sb[c][:, :],
                            start=st, stop=sp,
                        )
                        nc.tensor.matmul(
                            kp[:, :], lhsT=xt[c][:, :], rhs=wk_sb[c][:, :],
                            start=st, stop=sp,
                        )
                        nc.tensor.matmul(
                            vp[:, :], lhsT=xt[c][:, :], rhs=wv_sb[c][:, :],
                            start=st, stop=sp,
                        )

                    # V -> SBUF with a ones column per head (softmax sums)
                    nc.scalar.copy(
                        out=v_sb[i][:, :, 0:DK],
                        in_=vp[:, :].rearrange("p (h c) -> p h c", h=HG),
                    )
                    nc.scalar.copy(
                        out=v_sb[i][:, :, DK : DK + 1],
                        in_=onesf[:, :].rearrange("p (h c) -> p h c", c=1),
                    )

                    # RoPE on q/k (rotate-half layout: per head [32 even|32 odd])
                    cs3 = cs[:, :].rearrange("p (h c) -> p h c", h=HG)
                    sn3 = sn[:, :].rearrange("p (h c) -> p h c", h=HG)
                    for src, dst_tag in ((qp, "qr"), (kp, "kr")):
                        sv = src[:, :].rearrange(
                            "p (h t c) -> p h t c", h=HG, t=2
                        )
                        ev, od = sv[:, :, 0, :], sv[:, :, 1, :]
                        r = pas.tile([P, DG], MMD, tag=dst_tag, name=dst_tag)
                        rv = r[:, :].rearrange("p (h t c) -> p h t c", h=HG, t=2)
                        t1 = ropep.tile([P, HG * 32], F32, tag="t1")
                        t2 = ropep.tile([P, HG * 32], F32, tag="t2")
                        t13 = t1[:, :].rearrange("p (h c) -> p h c", h=HG)
                        t23 = t2[:, :].rearrange("p (h c) -> p h c", h=HG)
                        nc.vector.tensor_mul(t13, ev, cs3)
                        nc.vector.tensor_mul(t23, od, sn3)
                        nc.vector.tensor_sub(rv[:, :, 0, :], t13, t23)
                        t3 = ropep.tile([P, HG * 32], F32, tag="t3")
                        t4 = ropep.tile([P, HG * 32], F32, tag="t4")
                        t33 = t3[:, :].rearrange("p (h c) -> p h c", h=HG)
                        t43 = t4[:, :].rearrange("p (h c) -> p h c", h=HG)
                        nc.vector.tensor_mul(t33, ev, sn3)
                        nc.vector.tensor_mul(t43, od, cs3)
                        nc.vector.tensor_add(rv[:, :, 1, :], t33, t43)

                        # transpose head-pairs into [d, s] tiles
                        dst_tiles = qt_sb if dst_tag == "qr" else kt_sb
                        for p in range(4):
                            tp = tpp.tile([P, P], MMD, tag="tp")
                            nc.tensor.transpose(
                                tp[:, :], r[:, ts(p, P)], ident[:, :]
                            )
                            nc.scalar.copy(
                                out=dst_tiles[p][:, ts(i, P)], in_=tp[:, :]
                            )

            # ---------------- Phase B: attention per head ------------------
            with (
                tc.tile_pool(name="attn_psum", bufs=2, space="PSUM") as atp,
                tc.tile_pool(name="lg_psum", bufs=2, space="PSUM") as lgp,
                tc.tile_pool(name="pt_pool", bufs=3) as ptp,
                tc.tile_pool(name="sm_pool", bufs=2) as smp,
            ):
                for h in range(HG):
                    pair, poff = h // 2, 64 * (h % 2)
                    ap = atp.tile([DK + 1, S], F32, tag="attn")
                    for j in range(NS):
                        q0 = P * j
                        lq = S - q0
                        lg = lgp.tile([P, S], F32, tag="lg")
                        for qc in range(q0, S, 512):
                            n = min(512, S - qc)
                            nc.tensor.matmul(
                                lg[:, qc - q0 : qc - q0 + n],
                                lhsT=kt_sb[pair][poff : poff + DK, ts(j, P)],
                                rhs=qt_sb[pair][poff : poff + DK, qc : qc + n],
                                start=True, stop=True,
                            )
                        pt = ptp.tile([P, S], MMD, tag="pt")
                        nc.scalar.activation(
                            out=pt[:, 0:lq], in_=lg[:, 0:lq],
                            func=mybir.ActivationFunctionType.Exp,
                            scale=0.125,
                        )
                        nc.vector.tensor_mul(pt[:, 0:P], pt[:, 0:P], tril[:, :])
                        for c0 in range(0, S, 512):
                            lo = max(q0, c0)
                            hi = c0 + 512
                            if lo >= hi:
                                continue
                            nc.tensor.matmul(
                                ap[:, lo:hi],
                                lhsT=v_sb[j][:, h, :],
                                rhs=pt[:, lo - q0 : hi - q0],
                                start=(j == 0), stop=(j == NS - 1),
                                skip_group_check=True,
                            )
                    # softmax normalization: copy sums row to SBUF, reshape to
                    # [8,128] via a DRAM hop, reciprocal, then a partition-
                    # broadcast DMA read from DRAM; finally multiply.
                    sr = smp.tile([P, S], F32, tag="sr")
                    nc.scalar.copy(out=sr[64:65, :], in_=ap[DK : DK + 1, :])
                    nc.sync.dma_start(
                        out=rsum[h, :].rearrange("(o c) -> o c", o=1),
                        in_=sr[64:65, :],
                    )
                    rs8 = smp.tile([HG, P], F32, tag="rs8")
                    nc.sync.dma_start(
                        out=rs8[:, :],
                        in_=rsum[h, :].rearrange("(r c) -> r c", r=HG),
                    )
                    rc8 = smp.tile([HG, P], F32, tag="rc8")
                    nc.vector.reciprocal(out=rc8[:, :], in_=rs8[:, :])
                    nc.sync.dma_start(
                        out=rrec[h, :].rearrange("(r c) -> r c", r=HG),
                        in_=rc8[:, :],
                    )
                    row = rrec[h, :]
                    bc_src = bass.AP(
                        tensor=row.tensor, offset=row.offset, ap=[[0, DK], [1, S]]
                    )
                    bcs = smp.tile([DK, S], F32, tag="bcs")
                    nc.sync.dma_start(out=bcs[:, :], in_=bc_src)
                    if poff == 0:
                        nc.vector.tensor_mul(
                            at_sb[pair][0:DK, :], ap[0:DK, :], bcs[:, :]
                        )
                    else:
                        tmp = smp.tile([DK, S], MMD, tag="odd")
                        nc.vector.tensor_mul(tmp[:, :], ap[0:DK, :], bcs[:, :])
                        nc.sync.dma_start(
                            out=at_sb[pair][DK:P, :], in_=tmp[:, :]
                        )

            # ---------------- Phase D: output projection -------------------
            with (
                tc.tile_pool(name="y_psum", bufs=2, space="PSUM") as yp,
                tc.tile_pool(name="y_sbuf", bufs=3) as ys,
            ):
                for o in range(ND):
                    ypt = yp.tile([P, S], F32, tag="y")
                    for qc in range(0, S, 512):
                        for c in range(DG // P):
                            nc.tensor.matmul(
                                ypt[:, qc : qc + 512],
                                lhsT=wo_sb[c][:, ts(o, P)],
                                rhs=at_sb[c][:, qc : qc + 512],
                                start=(c == 0), stop=(c == DG // P - 1),
                            )
                    ysb = ys.tile([P, S], F32, tag="ysb")
                    nc.scalar.copy(out=ysb[:, :], in_=ypt[:, :])
                    nc.sync.dma_start(out=yT[ts(o, P), :], in_=ysb[:, :])

    _split_excess_waits(nc)
    return nc


_NC_CACHE = {}


def _get_nc():
    if "nc" not in _NC_CACHE:
        _NC_CACHE["nc"] = build_nc()
    return _NC_CACHE["nc"]


# rotate-half permutation within each head: evens then odds
_PERM = np.concatenate([np.arange(0, DK, 2), np.arange(1, DK, 2)])


def _host_prep(x, Wq, Wk, Wv, Wo, token_positions):
    """Build the 8 per-core input dicts."""
    inv_freq = 1.0 / (ROPE_THETA ** (np.arange(0, DK, 2, dtype=np.float32) / DK))
    in_maps = []
    for core in range(8):
        b, g = core // 2, core % 2
        heads = np.arange(HG * g, HG * (g + 1))
        rows_qk = (heads[:, None] * DK + _PERM[None, :]).reshape(-1)
        rows_v = (heads[:, None] * DK + np.arange(DK)[None, :]).reshape(-1)
        pos = token_positions[b].astype(np.float32)  # [S]
        ang = pos[:, None] * inv_freq[None, :]  # [S, 32]
        cos8 = np.tile(np.cos(ang), (1, HG)).astype(np.float32)
        sin8 = np.tile(np.sin(ang), (1, HG)).astype(np.float32)
        in_maps.append(
            {
                "xT": np.ascontiguousarray(x[b].T),
                "wqT": np.ascontiguousarray(Wq[rows_qk, :].T),
                "wkT": np.ascontiguousarray(Wk[rows_qk, :].T),
                "wvT": np.ascontiguousarray(Wv[rows_v, :].T),
                "woT": np.ascontiguousarray(Wo[:, rows_v].T),
                "cos8": cos8,
                "sin8": sin8,
            }
        )
    return in_maps


def kernel(x, Wq, Wk, Wv, Wo, token_positions, _trace=False):
    x = np.asarray(x, dtype=np.float32)
    Wq = np.asarray(Wq, dtype=np.float32)
    Wk = np.asarray(Wk, dtype=np.float32)
    Wv = np.asarray(Wv, dtype=np.float32)
    Wo = np.asarray(Wo, dtype=np.float32)
    token_positions = np.asarray(token_positions)

    nc = _get_nc()
    in_maps = _host_prep(x, Wq, Wk, Wv, Wo, token_positions)
    res = run_bass_kernel_spmd(nc, in_maps, core_ids=list(range(8)), trace=_trace)
    if _trace:
        kernel.last_exec_time_ns = res.exec_time_ns
        kernel.last_results = res

    y = np.empty((B, S, D), dtype=np.float32)
    for b in range(B):
        yT0 = res.results[2 * b]["yT"]
        yT1 = res.results[2 * b + 1]["yT"]
        y[b] = (yT0 + yT1).T
    return y


# revision 11
# speedup vs baseline: 2.0927x; 1.0655x over previous
"""Multi-head self-attention (B=4, S=1024, D=1024, H=16, RoPE, causal) on 8
Trainium2 NeuronCores.

Sharding: 8 cores = 4 batches x 2 head-groups (8 heads each). Each core
computes QKV projections for its batch/head-group, RoPE, causal attention,
and a partial output projection (contraction over its 512 attention dims).
The host sums the two partial outputs per batch (the "all-reduce") and
concatenates batches.

Device layout notes:
- Weights are passed transposed; Q/K projection output dims are permuted to
  rotate-half order (evens then odds within each head) so RoPE works on
  contiguous 32-column halves. Permuting Q and K identically leaves Q.K^T
  unchanged.
- Logits are computed transposed (L^T[k, q]) so softmax sums reduce over the
  PSUM partition axis via a ones-column appended to V, and the attention
  output arrives as attn^T[c, q] which feeds the output projection directly.
- Matmul operands are float32r (1.5 cyc/row vs 4 for fp32, ~1e-4 rel err).
- Softmax 1/sum is computed on a [8,128] reshape (DMA) and broadcast across
  partitions via a DRAM round-trip, keeping the PE out of it.
- The final output is produced transposed (y^T[o, q]); the host transposes.
"""

import numpy as np

import concourse.bass as bass
import concourse.mybir as mybir
import concourse.tile as tile
from concourse.bass import ts
from concourse.bass_utils import run_bass_kernel_spmd
from concourse.masks import make_identity, make_upper_triangular

B, S, D = 4, 1024, 1024
H = 16  # total heads
HG = 8  # heads per core (head-group)
DK = 64  # head dim
DG = HG * DK  # 512, per-core projection width
ROPE_THETA = 10000.0
P = 128  # partitions
NS = S // P  # 8 s-tiles
ND = D // P  # 8 d-chunks
F32 = mybir.dt.float32
F32R = mybir.dt.float32r

_uid = [0]


def _split_excess_waits(nc, limit=1):
    """This container's walrus rejects >1 sync waits on the kernel-tail
    Drain; move excess waits onto same-engine NoOps inserted before it."""
    for f in nc.m.functions:
        for blk in f.blocks:
            insts = list(blk.instructions)
            out = []
            changed = False
            for inst in insts:
                si = inst.sync_info
                if si is not None and si.on_wait and len(si.on_wait) > limit:
                    waits = list(si.on_wait)
                    head, tail = waits[:-limit], waits[-limit:]
                    for i in range(0, len(head), limit):
                        _uid[0] += 1
                        nop = mybir.InstNoOp(
                            name=f"waitsplit-{_uid[0]}", ins=[], outs=[]
                        )
                        nop.engine = inst.engine
                        nop.sync_info = mybir.SyncInfo(
                            on_wait=head[i : i + limit], on_update=[]
                        )
                        out.append(nop)
                    si.on_wait = tail
                    changed = True
                out.append(inst)
            if changed:
                blk.instructions = out
    return nc


def build_nc():
    nc = bass.Bass("TRN2")
    MMD = F32R  # matmul operand dtype
    xT = nc.dram_tensor("xT", [D, S], MMD, kind="ExternalInput")
    wqT = nc.dram_tensor("wqT", [D, DG], MMD, kind="ExternalInput")
    wkT = nc.dram_tensor("wkT", [D, DG], MMD, kind="ExternalInput")
    wvT = nc.dram_tensor("wvT", [D, DG], MMD, kind="ExternalInput")
    woT = nc.dram_tensor("woT", [DG, D], MMD, kind="ExternalInput")
    cos8 = nc.dram_tensor("cos8", [S, HG * 32], F32, kind="ExternalInput")
    sin8 = nc.dram_tensor("sin8", [S, HG * 32], F32, kind="ExternalInput")
    yT = nc.dram_tensor("yT", [D, S], F32, kind="ExternalOutput")
    # DRAM scratch for the softmax 1/sum partition-broadcast round-trip
    rsum = nc.dram_tensor("rsum", [HG, S], F32)
    rrec = nc.dram_tensor("rrec", [HG, S], F32)

    with tile.TileContext(nc) as tc:
        with (
            tc.tile_pool(name="const", bufs=1) as constp,
            tc.tile_pool(name="wq", bufs=1) as wqp,
            tc.tile_pool(name="big", bufs=1) as bigp,
        ):
            # constants (f32r tiles can't be memset directly; build in f32
            # and convert via ACT copy, which rounds)
            identf = constp.tile([P, P], F32, tag="identf")
            make_identity(nc, identf[:, :])
            ident = constp.tile([P, P], MMD, tag="ident")
            nc.scalar.copy(out=ident[:, :], in_=identf[:, :])
            tril = constp.tile([P, P], F32, tag="tril")
            make_upper_triangular(nc, tril[:, :], val=1.0, diag=True)
            onesf = constp.tile([P, HG], F32, tag="onesf")
            nc.vector.memset(onesf[:, :], 1.0)

            # resident weights (wo is loaded last: phase D needs it latest)
            wq_sb = [wqp.tile([P, DG], MMD, tag=f"wq{c}", name=f"wq{c}") for c in range(ND)]
            wk_sb = [wqp.tile([P, DG], MMD, tag=f"wk{c}", name=f"wk{c}") for c in range(ND)]
            wv_sb = [wqp.tile([P, DG], MMD, tag=f"wv{c}", name=f"wv{c}") for c in range(ND)]
            wo_sb = [wqp.tile([P, D], MMD, tag=f"wo{c}", name=f"wo{c}") for c in range(DG // P)]

            def load_weights():
                for c in range(ND):
                    nc.sync.dma_start(out=wq_sb[c][:, :], in_=wqT[ts(c, P), :])
                    nc.sync.dma_start(out=wk_sb[c][:, :], in_=wkT[ts(c, P), :])
                    nc.sync.dma_start(out=wv_sb[c][:, :], in_=wvT[ts(c, P), :])

            def load_wo():
                for c in range(DG // P):
                    nc.sync.dma_start(out=wo_sb[c][:, :], in_=woT[ts(c, P), :])

            # persistent activations
            qt_sb = [bigp.tile([P, S], MMD, tag=f"qt{p}", name=f"qt{p}") for p in range(4)]
            kt_sb = [bigp.tile([P, S], MMD, tag=f"kt{p}", name=f"kt{p}") for p in range(4)]
            v_sb = [bigp.tile([P, HG, DK + 1], MMD, tag=f"v{j}", name=f"v{j}") for j in range(NS)]
            at_sb = [bigp.tile([P, S], MMD, tag=f"at{p}", name=f"at{p}") for p in range(4)]

            # ---------------- Phase A: projections + RoPE + transposes ----
            with (
                tc.tile_pool(name="pa_psum", bufs=2, space="PSUM") as pap,
                tc.tile_pool(name="tp_psum", bufs=2, space="PSUM") as tpp,
                tc.tile_pool(name="pa_sbuf", bufs=3) as pas,
                tc.tile_pool(name="rope", bufs=4) as ropep,
            ):
                for i in range(NS):
                    xt = [pas.tile([P, P], MMD, tag="xt", name="xt") for _ in range(ND)]
                    for c in range(ND):
                        nc.sync.dma_start(
                            out=xt[c][:, :], in_=xT[ts(c, P), ts(i, P)]
                        )
                    cs = pas.tile([P, HG * 32], F32, tag="cos")
                    sn = pas.tile([P, HG * 32], F32, tag="sin")
                    nc.sync.dma_start(out=cs[:, :], in_=cos8[ts(i, P), :])
                    nc.sync.dma_start(out=sn[:, :], in_=sin8[ts(i, P), :])
                    if i == 0:
                        # emit weight loads after s-tile 0's x/cos/sin so the
                        # first projections aren't queued behind 8 MB of
                        # weight DMA
                        load_weights()

                    qp = pap.tile([P, DG], F32, tag="q")
                    kp = pap.tile([P, DG], F32, tag="k")
                    vp = pap.tile([P, DG], F32, tag="v")
                    for c in range(ND):
                        st = c == 0
                        sp = c == ND - 1
                        nc.tensor.matmul(
                            qp[:, :], lhsT=xt[c][:, :], rhs=wq_sb[c][:, :],
                            start=st, stop=sp,
                        )
                        nc.tensor.matmul(
                            kp[:, :], lhsT=xt[c][:, :], rhs=wk_sb[c][:, :],
                            start=st, stop=sp,
                        )
                        nc.tensor.matmul(
                            vp[:, :], lhsT=xt[c][:, :], rhs=wv_sb[c][:, :],
                            start=st, stop=sp,
                        )

                    # V -> SBUF with a ones column per head (softmax sums)
                    nc.scalar.copy(
                        out=v_sb[i][:, :, 0:DK],
                        in_=vp[:, :].rearrange("p (h c) -> p h c", h=HG),
                    )
                    nc.scalar.copy(
                        out=v_sb[i][:, :, DK : DK + 1],
                        in_=onesf[:, :].rearrange("p (h c) -> p h c", c=1),
                    )

                    # RoPE on q/k (rotate-half layout: per head [32 even|32 odd])
                    cs3 = cs[:, :].rearrange("p (h c) -> p h c", h=HG)
                    sn3 = sn[:, :].rearrange("p (h c) -> p h c", h=HG)
                    for src, dst_tag in ((qp, "qr"), (kp, "kr")):
                        sv = src[:, :].rearrange(
                            "p (h t c) -> p h t c", h=HG, t=2
                        )
                        ev, od = sv[:, :, 0, :], sv[:, :, 1, :]
                        r = pas.tile([P, DG], MMD, tag=dst_tag, name=dst_tag)
                        rv = r[:, :].rearrange("p (h t c) -> p h t c", h=HG, t=2)
                        t1 = ropep.tile([P, HG * 32], F32, tag="t1")
                        t2 = ropep.tile([P, HG * 32], F32, tag="t2")
                        t13 = t1[:, :].rearrange("p (h c) -> p h c", h=HG)
                        t23 = t2[:, :].rearrange("p (h c) -> p h c", h=HG)
                        nc.vector.tensor_mul(t13, ev, cs3)
                        nc.vector.tensor_mul(t23, od, sn3)
                        nc.vector.tensor_sub(rv[:, :, 0, :], t13, t23)
                        t3 = ropep.tile([P, HG * 32], F32, tag="t3")
                        t4 = ropep.tile([P, HG * 32], F32, tag="t4")
                        t33 = t3[:, :].rearrange("p (h c) -> p h c", h=HG)
                        t43 = t4[:, :].rearrange("p (h c) -> p h c", h=HG)
                        nc.vector.tensor_mul(t33, ev, sn3)
                        nc.vector.tensor_mul(t43, od, cs3)
                        nc.vector.tensor_add(rv[:, :, 1, :], t33, t43)

                        # transpose head-pairs into [d, s] tiles
                        dst_tiles = qt_sb if dst_tag == "qr" else kt_sb
                        for p in range(4):
                            tp = tpp.tile([P, P], MMD, tag="tp")
                            nc.tensor.transpose(
                                tp[:, :], r[:, ts(p, P)], ident[:, :]
                            )
                            nc.scalar.copy(
                                out=dst_tiles[p][:, ts(i, P)], in_=tp[:, :]
                            )

            # ---------------- Phase B: attention per head ------------------
            with (
                tc.tile_pool(name="attn_psum", bufs=2, space="PSUM") as atp,
                tc.tile_pool(name="lg_psum", bufs=2, space="PSUM") as lgp,
                tc.tile_pool(name="pt_pool", bufs=3) as ptp,
                tc.tile_pool(name="sm_pool", bufs=2) as smp,
            ):
                load_wo()

                def emit_ev(ap, j, pt):
                    q0 = P * j
                    for c0 in range(0, S, 512):
                        lo = max(q0, c0)
                        hi = c0 + 512
                        if lo >= hi:
                            continue
                        nc.tensor.matmul(
                            ap[:, lo:hi],
                            lhsT=v_sb[j][:, h, :],
                            rhs=pt[:, lo - q0 : hi - q0],
                            start=(j == 0), stop=(j == NS - 1),
                            skip_group_check=True,
                        )

                for h in range(HG):
                    pair, poff = h // 2, 64 * (h % 2)
                    ap = atp.tile([DK + 1, S], F32, tag="attn")
                    pending = None
                    for j in range(NS):
                        q0 = P * j
                        lq = S - q0
                        lg = lgp.tile([P, S], F32, tag="lg")
                        for qc in range(q0, S, 512):
                            n = min(512, S - qc)
                            nc.tensor.matmul(
                                lg[:, qc - q0 : qc - q0 + n],
                                lhsT=kt_sb[pair][poff : poff + DK, ts(j, P)],
                                rhs=qt_sb[pair][poff : poff + DK, qc : qc + n],
                                start=True, stop=True,
                            )
                        pt = ptp.tile([P, S], MMD, tag="pt")
                        nc.scalar.activation(
                            out=pt[:, 0:lq], in_=lg[:, 0:lq],
                            func=mybir.ActivationFunctionType.Exp,
                            scale=0.125,
                        )
                        nc.vector.tensor_mul(pt[:, 0:P], pt[:, 0:P], tril[:, :])
                        # software pipeline: emit EV(j-1) after QK(j) so the
                        # PE never stalls waiting for exp(j)
                        if pending is not None:
                            emit_ev(ap, *pending)
                        pending = (j, pt)
                    emit_ev(ap, *pending)
                    # softmax normalization: copy sums row to SBUF, reshape to
                    # [8,128] via a DRAM hop, reciprocal, then a partition-
                    # broadcast DMA read from DRAM; finally multiply.
                    sr = smp.tile([P, S], F32, tag="sr")
                    nc.scalar.copy(out=sr[64:65, :], in_=ap[DK : DK + 1, :])
                    nc.sync.dma_start(
                        out=rsum[h, :].rearrange("(o c) -> o c", o=1),
                        in_=sr[64:65, :],
                    )
                    rs8 = smp.tile([HG, P], F32, tag="rs8")
                    nc.sync.dma_start(
                        out=rs8[:, :],
                        in_=rsum[h, :].rearrange("(r c) -> r c", r=HG),
                    )
                    rc8 = smp.tile([HG, P], F32, tag="rc8")
                    nc.vector.reciprocal(out=rc8[:, :], in_=rs8[:, :])
                    nc.sync.dma_start(
                        out=rrec[h, :].rearrange("(r c) -> r c", r=HG),
                        in_=rc8[:, :],
                    )
                    row = rrec[h, :]
                    bc_src = bass.AP(
                        tensor=row.tensor, offset=row.offset, ap=[[0, DK], [1, S]]
                    )
                    bcs = smp.tile([DK, S], F32, tag="bcs")
                    nc.sync.dma_start(out=bcs[:, :], in_=bc_src)
                    if poff == 0:
                        nc.vector.tensor_mul(
                            at_sb[pair][0:DK, :], ap[0:DK, :], bcs[:, :]
                        )
                    else:
                        tmp = smp.tile([DK, S], MMD, tag="odd")
                        nc.vector.tensor_mul(tmp[:, :], ap[0:DK, :], bcs[:, :])
                        nc.sync.dma_start(
                            out=at_sb[pair][DK:P, :], in_=tmp[:, :]
                        )

            # ---------------- Phase D: output projection -------------------
            with (
                tc.tile_pool(name="y_psum", bufs=2, space="PSUM") as yp,
                tc.tile_pool(name="y_sbuf", bufs=3) as ys,
            ):
                for o in range(ND):
                    ypt = yp.tile([P, S], F32, tag="y")
                    for qc in range(0, S, 512):
                        for c in range(DG // P):
                            nc.tensor.matmul(
                                ypt[:, qc : qc + 512],
                                lhsT=wo_sb[c][:, ts(o, P)],
                                rhs=at_sb[c][:, qc : qc + 512],
                                start=(c == 0), stop=(c == DG // P - 1),
                            )
                    ysb = ys.tile([P, S], F32, tag="ysb")
                    nc.scalar.copy(out=ysb[:, :], in_=ypt[:, :])
                    nc.sync.dma_start(out=yT[ts(o, P), :], in_=ysb[:, :])

    _split_excess_waits(nc)
    return nc


_NC_CACHE = {}


def _get_nc():
    if "nc" not in _NC_CACHE:
        _NC_CACHE["nc"] = build_nc()
    return _NC_CACHE["nc"]


# rotate-half permutation within each head: evens then odds
_PERM = np.concatenate([np.arange(0, DK, 2), np.arange(1, DK, 2)])


def _host_prep(x, Wq, Wk, Wv, Wo, token_positions):
    """Build the 8 per-core input dicts."""
    inv_freq = 1.0 / (ROPE_THETA ** (np.arange(0, DK, 2, dtype=np.float32) / DK))
    in_maps = []
    for core in range(8):
        b, g = core // 2, core % 2
        heads = np.arange(HG * g, HG * (g + 1))
        rows_qk = (heads[:, None] * DK + _PERM[None, :]).reshape(-1)
        rows_v = (heads[:, None] * DK + np.arange(DK)[None, :]).reshape(-1)
        pos = token_positions[b].astype(np.float32)  # [S]
        ang = pos[:, None] * inv_freq[None, :]  # [S, 32]
        cos8 = np.tile(np.cos(ang), (1, HG)).astype(np.float32)
        sin8 = np.tile(np.sin(ang), (1, HG)).astype(np.float32)
        in_maps.append(
            {
                "xT": np.ascontiguousarray(x[b].T),
                "wqT": np.ascontiguousarray(Wq[rows_qk, :].T),
                "wkT": np.ascontiguousarray(Wk[rows_qk, :].T),
                "wvT": np.ascontiguousarray(Wv[rows_v, :].T),
                "woT": np.ascontiguousarray(Wo[:, rows_v].T),
                "cos8": cos8,
                "sin8": sin8,
            }
        )
    return in_maps


def kernel(x, Wq, Wk, Wv, Wo, token_positions, _trace=False):
    x = np.asarray(x, dtype=np.float32)
    Wq = np.asarray(Wq, dtype=np.float32)
    Wk = np.asarray(Wk, dtype=np.float32)
    Wv = np.asarray(Wv, dtype=np.float32)
    Wo = np.asarray(Wo, dtype=np.float32)
    token_positions = np.asarray(token_positions)

    nc = _get_nc()
    in_maps = _host_prep(x, Wq, Wk, Wv, Wo, token_positions)
    res = run_bass_kernel_spmd(nc, in_maps, core_ids=list(range(8)), trace=_trace)
    if _trace:
        kernel.last_exec_time_ns = res.exec_time_ns
        kernel.last_results = res

    y = np.empty((B, S, D), dtype=np.float32)
    for b in range(B):
        yT0 = res.results[2 * b]["yT"]
        yT1 = res.results[2 * b + 1]["yT"]
        y[b] = (yT0 + yT1).T
    return y


# revision 13
# speedup vs baseline: 2.4151x; 1.1540x over previous
"""Multi-head self-attention (B=4, S=1024, D=1024, H=16, RoPE, causal) on 8
Trainium2 NeuronCores.

Sharding: 8 cores = 4 batches x 2 head-groups (8 heads each). Each core
computes QKV projections for its batch/head-group, RoPE, causal attention,
and a partial output projection (contraction over its 512 attention dims).
The host sums the two partial outputs per batch (the "all-reduce") and
concatenates batches.

Device layout notes:
- Weights are passed transposed; Q/K projection output dims are permuted to
  rotate-half order (evens then odds within each head) so RoPE works on
  contiguous 32-column halves. Permuting Q and K identically leaves Q.K^T
  unchanged.
- Logits are computed transposed (L^T[k, q]) so softmax sums reduce over the
  PSUM partition axis via a ones-column appended to V, and the attention
  output arrives as attn^T[c, q] which feeds the output projection directly.
- Matmul operands are float32r (1.5 cyc/row vs 4 for fp32, ~1e-4 rel err).
- Softmax 1/sum is computed on a [8,128] reshape (DMA) and broadcast across
  partitions via a DRAM round-trip, keeping the PE out of it.
- The final output is produced transposed (y^T[o, q]); the host transposes.
"""

import numpy as np

import concourse.bass as bass
import concourse.mybir as mybir
import concourse.tile as tile
from concourse.bass import ts
from concourse.bass_utils import run_bass_kernel_spmd
from concourse.masks import make_identity, make_upper_triangular

B, S, D = 4, 1024, 1024
H = 16  # total heads
HG = 8  # heads per core (head-group)
DK = 64  # head dim
DG = HG * DK  # 512, per-core projection width
ROPE_THETA = 10000.0
P = 128  # partitions
NS = S // P  # 8 s-tiles
ND = D // P  # 8 d-chunks
F32 = mybir.dt.float32
F32R = mybir.dt.float32r

_uid = [0]


def _split_excess_waits(nc, limit=1):
    """This container's walrus rejects >1 sync waits on the kernel-tail
    Drain; move excess waits onto same-engine NoOps inserted before it."""
    for f in nc.m.functions:
        for blk in f.blocks:
            insts = list(blk.instructions)
            out = []
            changed = False
            for inst in insts:
                si = inst.sync_info
                if si is not None and si.on_wait and len(si.on_wait) > limit:
                    waits = list(si.on_wait)
                    head, tail = waits[:-limit], waits[-limit:]
                    for i in range(0, len(head), limit):
                        _uid[0] += 1
                        nop = mybir.InstNoOp(
                            name=f"waitsplit-{_uid[0]}", ins=[], outs=[]
                        )
                        nop.engine = inst.engine
                        nop.sync_info = mybir.SyncInfo(
                            on_wait=head[i : i + limit], on_update=[]
                        )
                        out.append(nop)
                    si.on_wait = tail
                    changed = True
                out.append(inst)
            if changed:
                blk.instructions = out
    return nc


def build_nc():
    nc = bass.Bass("TRN2")
    MMD = F32R  # matmul operand dtype
    xT = nc.dram_tensor("xT", [D, S], MMD, kind="ExternalInput")
    wqT = nc.dram_tensor("wqT", [D, DG], MMD, kind="ExternalInput")
    wkT = nc.dram_tensor("wkT", [D, DG], MMD, kind="ExternalInput")
    wvT = nc.dram_tensor("wvT", [D, DG], MMD, kind="ExternalInput")
    woT = nc.dram_tensor("woT", [DG, D], MMD, kind="ExternalInput")
    cos8 = nc.dram_tensor("cos8", [S, HG * 32], F32, kind="ExternalInput")
    sin8 = nc.dram_tensor("sin8", [S, HG * 32], F32, kind="ExternalInput")
    yT = nc.dram_tensor("yT", [D, S], F32, kind="ExternalOutput")
    # DRAM scratch for the softmax 1/sum partition-broadcast round-trip
    rsum = nc.dram_tensor("rsum", [HG, S], F32)
    rrec = nc.dram_tensor("rrec", [HG, S], F32)

    with tile.TileContext(nc) as tc:
        with (
            tc.tile_pool(name="const", bufs=1) as constp,
            tc.tile_pool(name="wq", bufs=1) as wqp,
            tc.tile_pool(name="big", bufs=1) as bigp,
        ):
            # constants (f32r tiles can't be memset directly; build in f32
            # and convert via ACT copy, which rounds)
            identf = constp.tile([P, P], F32, tag="identf")
            make_identity(nc, identf[:, :])
            ident = constp.tile([P, P], MMD, tag="ident")
            nc.scalar.copy(out=ident[:, :], in_=identf[:, :])
            tril = constp.tile([P, P], F32, tag="tril")
            make_upper_triangular(nc, tril[:, :], val=1.0, diag=True)
            onesf = constp.tile([P, HG], F32, tag="onesf")
            nc.vector.memset(onesf[:, :], 1.0)

            # resident weights, one batched tile+DMA per weight
            # (wo is loaded at the start of phase B: phase D needs it latest)
            wq_all = wqp.tile([P, ND, DG], MMD, tag="wq", name="wq_all")
            wk_all = wqp.tile([P, ND, DG], MMD, tag="wk", name="wk_all")
            wv_all = wqp.tile([P, ND, DG], MMD, tag="wv", name="wv_all")
            wo_all = wqp.tile([P, DG // P, D], MMD, tag="wo", name="wo_all")
            wq_sb = [wq_all[:, c, :] for c in range(ND)]
            wk_sb = [wk_all[:, c, :] for c in range(ND)]
            wv_sb = [wv_all[:, c, :] for c in range(ND)]
            wo_sb = [wo_all[:, c, :] for c in range(DG // P)]

            def load_weights():
                for w_all, wT in ((wq_all, wqT), (wk_all, wkT), (wv_all, wvT)):
                    nc.sync.dma_start(
                        out=w_all[:, :, :],
                        in_=wT[:, :].rearrange("(c p) o -> p c o", p=P),
                    )

            def load_wo():
                nc.sync.dma_start(
                    out=wo_all[:, :, :],
                    in_=woT[:, :].rearrange("(c p) o -> p c o", p=P),
                )

            # persistent activations
            qt_sb = [bigp.tile([P, S], MMD, tag=f"qt{p}", name=f"qt{p}") for p in range(4)]
            kt_sb = [bigp.tile([P, S], MMD, tag=f"kt{p}", name=f"kt{p}") for p in range(4)]
            v_sb = [bigp.tile([P, HG, DK + 1], MMD, tag=f"v{j}", name=f"v{j}") for j in range(NS)]
            at_sb = [bigp.tile([P, S], MMD, tag=f"at{p}", name=f"at{p}") for p in range(4)]

            # ---------------- Phase A: projections + RoPE + transposes ----
            with (
                tc.tile_pool(name="pa_psum", bufs=2, space="PSUM") as pap,
                tc.tile_pool(name="tp_psum", bufs=2, space="PSUM") as tpp,
                tc.tile_pool(name="pa_sbuf", bufs=3) as pas,
                tc.tile_pool(name="rope", bufs=4) as ropep,
            ):
                for i in range(NS):
                    xt_all = pas.tile([P, ND, P], MMD, tag="xt", name="xt")
                    nc.sync.dma_start(
                        out=xt_all[:, :, :],
                        in_=xT[:, :].rearrange("(c p) s -> p c s", p=P)[
                            :, :, ts(i, P)
                        ],
                    )
                    xt = [xt_all[:, c, :] for c in range(ND)]
                    cs = pas.tile([P, HG * 32], F32, tag="cos")
                    sn = pas.tile([P, HG * 32], F32, tag="sin")
                    nc.sync.dma_start(out=cs[:, :], in_=cos8[ts(i, P), :])
                    nc.sync.dma_start(out=sn[:, :], in_=sin8[ts(i, P), :])
                    if i == 0:
                        # emit weight loads after s-tile 0's x/cos/sin so the
                        # first projections aren't queued behind 8 MB of
                        # weight DMA
                        load_weights()

                    qp = pap.tile([P, DG], F32, tag="q")
                    kp = pap.tile([P, DG], F32, tag="k")
                    vp = pap.tile([P, DG], F32, tag="v")
                    for c in range(ND):
                        st = c == 0
                        sp = c == ND - 1
                        nc.tensor.matmul(
                            qp[:, :], lhsT=xt[c], rhs=wq_sb[c],
                            start=st, stop=sp,
                        )
                        nc.tensor.matmul(
                            kp[:, :], lhsT=xt[c], rhs=wk_sb[c],
                            start=st, stop=sp,
                        )
                        nc.tensor.matmul(
                            vp[:, :], lhsT=xt[c], rhs=wv_sb[c],
                            start=st, stop=sp,
                        )

                    # V -> SBUF with a ones column per head (softmax sums)
                    nc.scalar.copy(
                        out=v_sb[i][:, :, 0:DK],
                        in_=vp[:, :].rearrange("p (h c) -> p h c", h=HG),
                    )
                    nc.scalar.copy(
                        out=v_sb[i][:, :, DK : DK + 1],
                        in_=onesf[:, :].rearrange("p (h c) -> p h c", c=1),
                    )

                    # RoPE on q/k (rotate-half layout: per head [32 even|32 odd])
                    cs3 = cs[:, :].rearrange("p (h c) -> p h c", h=HG)
                    sn3 = sn[:, :].rearrange("p (h c) -> p h c", h=HG)
                    for src, dst_tag in ((qp, "qr"), (kp, "kr")):
                        sv = src[:, :].rearrange(
                            "p (h t c) -> p h t c", h=HG, t=2
                        )
                        ev, od = sv[:, :, 0, :], sv[:, :, 1, :]
                        r = pas.tile([P, DG], MMD, tag=dst_tag, name=dst_tag)
                        rv = r[:, :].rearrange("p (h t c) -> p h t c", h=HG, t=2)
                        t1 = ropep.tile([P, HG * 32], F32, tag="t1")
                        t2 = ropep.tile([P, HG * 32], F32, tag="t2")
                        t13 = t1[:, :].rearrange("p (h c) -> p h c", h=HG)
                        t23 = t2[:, :].rearrange("p (h c) -> p h c", h=HG)
                        nc.vector.tensor_mul(t13, ev, cs3)
                        nc.vector.tensor_mul(t23, od, sn3)
                        nc.vector.tensor_sub(rv[:, :, 0, :], t13, t23)
                        t3 = ropep.tile([P, HG * 32], F32, tag="t3")
                        t4 = ropep.tile([P, HG * 32], F32, tag="t4")
                        t33 = t3[:, :].rearrange("p (h c) -> p h c", h=HG)
                        t43 = t4[:, :].rearrange("p (h c) -> p h c", h=HG)
                        nc.vector.tensor_mul(t33, ev, sn3)
                        nc.vector.tensor_mul(t43, od, cs3)
                        nc.vector.tensor_add(rv[:, :, 1, :], t33, t43)

                        # transpose head-pairs into [d, s] tiles
                        dst_tiles = qt_sb if dst_tag == "qr" else kt_sb
                        for p in range(4):
                            tp = tpp.tile([P, P], MMD, tag="tp")
                            nc.tensor.transpose(
                                tp[:, :], r[:, ts(p, P)], ident[:, :]
                            )
                            nc.scalar.copy(
                                out=dst_tiles[p][:, ts(i, P)], in_=tp[:, :]
                            )

            # ---------------- Phase B: attention per head ------------------
            with (
                tc.tile_pool(name="attn_psum", bufs=2, space="PSUM") as atp,
                tc.tile_pool(name="lg_psum", bufs=2, space="PSUM") as lgp,
                tc.tile_pool(name="pt_pool", bufs=3) as ptp,
                tc.tile_pool(name="sm_pool", bufs=2) as smp,
            ):
                load_wo()

                def emit_ev(ap, j, pt):
                    q0 = P * j
                    for c0 in range(0, S, 512):
                        lo = max(q0, c0)
                        hi = c0 + 512
                        if lo >= hi:
                            continue
                        nc.tensor.matmul(
                            ap[:, lo:hi],
                            lhsT=v_sb[j][:, h, :],
                            rhs=pt[:, lo - q0 : hi - q0],
                            start=(j == 0), stop=(j == NS - 1),
                            skip_group_check=True,
                        )

                for h in range(HG):
                    pair, poff = h // 2, 64 * (h % 2)
                    ap = atp.tile([DK + 1, S], F32, tag="attn")
                    pending = None
                    for j in range(NS):
                        q0 = P * j
                        lq = S - q0
                        lg = lgp.tile([P, S], F32, tag="lg")
                        for qc in range(q0, S, 512):
                            n = min(512, S - qc)
                            nc.tensor.matmul(
                                lg[:, qc - q0 : qc - q0 + n],
                                lhsT=kt_sb[pair][poff : poff + DK, ts(j, P)],
                                rhs=qt_sb[pair][poff : poff + DK, qc : qc + n],
                                start=True, stop=True,
                            )
                        pt = ptp.tile([P, S], MMD, tag="pt")
                        nc.scalar.activation(
                            out=pt[:, 0:lq], in_=lg[:, 0:lq],
                            func=mybir.ActivationFunctionType.Exp,
                            scale=0.125,
                        )
                        nc.vector.tensor_mul(pt[:, 0:P], pt[:, 0:P], tril[:, :])
                        # software pipeline: emit EV(j-1) after QK(j) so the
                        # PE never stalls waiting for exp(j)
                        if pending is not None:
                            emit_ev(ap, *pending)
                        pending = (j, pt)
                    emit_ev(ap, *pending)
                    # softmax normalization: copy sums row to SBUF, reshape to
                    # [8,128] via a DRAM hop, reciprocal, then a partition-
                    # broadcast DMA read from DRAM; finally multiply.
                    sr = smp.tile([P, S], F32, tag="sr")
                    nc.scalar.copy(out=sr[64:65, :], in_=ap[DK : DK + 1, :])
                    nc.sync.dma_start(
                        out=rsum[h, :].rearrange("(o c) -> o c", o=1),
                        in_=sr[64:65, :],
                    )
                    rs8 = smp.tile([HG, P], F32, tag="rs8")
                    nc.sync.dma_start(
                        out=rs8[:, :],
                        in_=rsum[h, :].rearrange("(r c) -> r c", r=HG),
                    )
                    rc8 = smp.tile([HG, P], F32, tag="rc8")
                    nc.vector.reciprocal(out=rc8[:, :], in_=rs8[:, :])
                    nc.sync.dma_start(
                        out=rrec[h, :].rearrange("(r c) -> r c", r=HG),
                        in_=rc8[:, :],
                    )
                    row = rrec[h, :]
                    bc_src = bass.AP(
                        tensor=row.tensor, offset=row.offset, ap=[[0, DK], [1, S]]
                    )
                    bcs = smp.tile([DK, S], F32, tag="bcs")
                    nc.sync.dma_start(out=bcs[:, :], in_=bc_src)
                    if poff == 0:
                        nc.vector.tensor_mul(
                            at_sb[pair][0:DK, :], ap[0:DK, :], bcs[:, :]
                        )
                    else:
                        tmp = smp.tile([DK, S], MMD, tag="odd")
                        nc.vector.tensor_mul(tmp[:, :], ap[0:DK, :], bcs[:, :])
                        nc.sync.dma_start(
                            out=at_sb[pair][DK:P, :], in_=tmp[:, :]
                        )

            # ---------------- Phase D: output projection -------------------
            with (
                tc.tile_pool(name="y_psum", bufs=2, space="PSUM") as yp,
                tc.tile_pool(name="y_sbuf", bufs=3) as ys,
            ):
                for o in range(ND):
                    ypt = yp.tile([P, S], F32, tag="y")
                    for qc in range(0, S, 512):
                        for c in range(DG // P):
                            nc.tensor.matmul(
                                ypt[:, qc : qc + 512],
                                lhsT=wo_sb[c][:, ts(o, P)],
                                rhs=at_sb[c][:, qc : qc + 512],
                                start=(c == 0), stop=(c == DG // P - 1),
                            )
                    ysb = ys.tile([P, S], F32, tag="ysb")
                    nc.scalar.copy(out=ysb[:, :], in_=ypt[:, :])
                    nc.sync.dma_start(out=yT[ts(o, P), :], in_=ysb[:, :])

    _split_excess_waits(nc)
    return nc


_NC_CACHE = {}


def _get_nc():
    if "nc" not in _NC_CACHE:
        _NC_CACHE["nc"] = build_nc()
    return _NC_CACHE["nc"]


# rotate-half permutation within each head: evens then odds
_PERM = np.concatenate([np.arange(0, DK, 2), np.arange(1, DK, 2)])


def _host_prep(x, Wq, Wk, Wv, Wo, token_positions):
    """Build the 8 per-core input dicts."""
    inv_freq = 1.0 / (ROPE_THETA ** (np.arange(0, DK, 2, dtype=np.float32) / DK))
    in_maps = []
    for core in range(8):
        b, g = core // 2, core % 2
        heads = np.arange(HG * g, HG * (g + 1))
        rows_qk = (heads[:, None] * DK + _PERM[None, :]).reshape(-1)
        rows_v = (heads[:, None] * DK + np.arange(DK)[None, :]).reshape(-1)
        pos = token_positions[b].astype(np.float32)  # [S]
        ang = pos[:, None] * inv_freq[None, :]  # [S, 32]
        cos8 = np.tile(np.cos(ang), (1, HG)).astype(np.float32)
        sin8 = np.tile(np.sin(ang), (1, HG)).astype(np.float32)
        in_maps.append(
            {
                "xT": np.ascontiguousarray(x[b].T),
                "wqT": np.ascontiguousarray(Wq[rows_qk, :].T),
                "wkT": np.ascontiguousarray(Wk[rows_qk, :].T),
                "wvT": np.ascontiguousarray(Wv[rows_v, :].T),
                "woT": np.ascontiguousarray(Wo[:, rows_v].T),
                "cos8": cos8,
                "sin8": sin8,
            }
        )
    return in_maps


def kernel(x, Wq, Wk, Wv, Wo, token_positions, _trace=False):
    x = np.asarray(x, dtype=np.float32)
    Wq = np.asarray(Wq, dtype=np.float32)
    Wk = np.asarray(Wk, dtype=np.float32)
    Wv = np.asarray(Wv, dtype=np.float32)
    Wo = np.asarray(Wo, dtype=np.float32)
    token_positions = np.asarray(token_positions)

    nc = _get_nc()
    in_maps = _host_prep(x, Wq, Wk, Wv, Wo, token_positions)
    res = run_bass_kernel_spmd(nc, in_maps, core_ids=list(range(8)), trace=_trace)
    if _trace:
        kernel.last_exec_time_ns = res.exec_time_ns
        kernel.last_results = res

    y = np.empty((B, S, D), dtype=np.float32)
    for b in range(B):
        yT0 = res.results[2 * b]["yT"]
        yT1 = res.results[2 * b + 1]["yT"]
        y[b] = (yT0 + yT1).T
    return y
